# revision 15
# baseline (speedup 1.0000x reference)
"""Trainium2 Bass kernel for a 2-layer GCN (PyG GCNConv semantics) on 8 NeuronCores.

Strategy (dst-sharding):
  - nodes sharded 12500/core (padded to 12544 = 98*128 tiles of 128 rows)
  - all node-row tensors live in DRAM in a block-major layout (row
    bm = block_base + r*TB + j for tile j of the block): per-BLOCK 256KB
    contiguous dma_starts replace per-tile 32KB ones, and gather indices
    are emitted against the same layout
  - edges (self-loops excluded) partitioned by destination core; within a
    core grouped by (dst-tile, src-group) where a src group is 2 source
    cores (25088 rows < int16 gather-index limit), padded to 128-edge chunks
  - per chunk: dma_gather of 128 source rows (fp16, round-robin over 4
    SWDGE queues) + pure one-hot P[t,r]=(iota==dr) built 8 chunks per
    tensor_tensor (broadcast-AP read of per-chunk dst-row columns); a PE
    matmul accumulates agg^T into PSUM
  - self-loop term: per-block contiguous load + per-tile matmul against a
    constant identity (no gather rows spent on it)
  - per dst-tile: agg^T @ W^T flips orientation back to [row, feat];
    dinv[dst] (squared for layer 1, which also pre-scales the next layer's
    gather source) is applied in the epilogue as a per-partition scalar
  - layer 1 -> AllGather of y1 shards (block-major) -> layer 2
"""
import sys

sys.path.insert(0, "/opt/trn_rl_repo")

import numpy as np

N = 100000
E = 1600000
D = 128
CORES = 8
S = 12500          # real nodes per core
TPC = 98           # dst tiles per core
SP = TPC * 128     # padded nodes per core (12544)
NP = CORES * SP    # padded global rows (100352)
NGRP = 4           # src groups = pairs of cores
GRPW = 2 * SP      # rows per src group (25088 < 32768 int16 limit)
BLK = 8            # dst tiles per block
NB = (TPC + BLK - 1) // BLK  # 13 blocks (12 full + 1 of 2 tiles)
GSPLIT = 24        # max columns (128-idx chunks) per gather instruction
NQ = 4             # SWDGE queues, gathers round-robin across them


def _tiles_in_block(b):
    return BLK if b < TPC // BLK else TPC - (TPC // BLK) * BLK


def _bm_row(l):
    """Local row index -> block-major row index (vectorized)."""
    l = np.asarray(l)
    t = l >> 7
    r = l & 127
    b = t // BLK
    j = t - b * BLK
    tb = np.where(b < TPC // BLK, BLK, TPC - (TPC // BLK) * BLK)
    return b * BLK * 128 + r * tb + j


def _build_schedule(src, dst):
    """Static chunk schedule shared by all cores (SPMD: one instruction
    stream). Returns per-core slot arrays + the chunk/block layout."""
    core = dst // S
    dl = dst % S
    t = dl >> 7
    r = dl & 127
    score = src // S
    g = score // 2
    srel = (score % 2) * SP + _bm_row(src % S)

    key = (core * TPC + t) * NGRP + g
    order = np.argsort(key, kind="stable")
    cnt = np.bincount(key, minlength=CORES * TPC * NGRP).reshape(CORES, TPC, NGRP)
    K = -(-cnt.max(0) // 128)  # [TPC, NGRP] chunks per (tile, group)

    # gather/slot layout order: for b in blocks: for g: for t in b: for k in
    # K[t,g].  Chunk IDs (dr-table columns, P-build batches) are assigned
    # separately in CONSUMPTION order (for b: for t: for g: for k) so the
    # lazily-built 8-chunk P batches are created and retired in the same
    # order PE consumes them — a first-use ordering would invert against
    # the in-order Vector queue and deadlock the tile-pool rings.
    chunk_start = np.zeros((TPC, NGRP), np.int64)
    blocks = []
    nchunks = 0
    raw_chunks = {}  # (t, g, k) -> gather-order chunk index (slot base / m col)
    for b in range(NB):
        tiles = list(range(b * BLK, min((b + 1) * BLK, TPC)))
        col = 0
        gathers = []
        for gg in range(NGRP):
            for tt in tiles:
                chunk_start[tt, gg] = nchunks
                c0 = col
                s0 = nchunks * 128
                for k in range(int(K[tt, gg])):
                    raw_chunks[(tt, gg, k)] = (col, nchunks)
                    col += 1
                    nchunks += 1
                # one gather piece per (tile, group): each core's dead slots
                # (its padding up to the shared chunk count) then sit at the
                # END of the piece and carry idx=-1, which the gather engine
                # skips ("negative indices at the end are ignored") — the
                # padding costs no DMA.  Pieces round-robin the SWDGE queues.
                if col > c0:
                    gathers.append((gg, c0, col, s0, (col - c0) * 128))
        blocks.append(dict(tiles=tiles, C=col, gathers=gathers))
    NC = nchunks
    NSLOT = NC * 128

    # consumption-ordered chunk ids + gather-chunk -> chid permutation
    chid_of_gc = np.zeros(NC, np.int64)
    cid = 0
    for b, blk in enumerate(blocks):
        tile_chunks = {}
        for tt in blk["tiles"]:
            lst = []
            for gg in range(NGRP):
                for k in range(int(K[tt, gg])):
                    col, gc = raw_chunks[(tt, gg, k)]
                    chid_of_gc[gc] = cid
                    lst.append((col, cid))
                    cid += 1
            tile_chunks[tt] = lst
        blk["chunks"] = tile_chunks

    # per-core slot arrays
    skey = key[order]
    runs = np.flatnonzero(np.diff(skey)) + 1
    starts = np.r_[0, runs]
    lens = np.diff(np.r_[starts, len(skey)])
    pos = np.arange(len(skey)) - np.repeat(starts, lens)
    slot = chunk_start[t[order], g[order]] * 128 + pos

    idx_slot = np.zeros((CORES, NSLOT), np.int16)
    dr_slot = np.full((CORES, NSLOT), -1.0, np.float32)
    co = core[order]
    idx_slot[co, slot] = srel[order].astype(np.int16)
    dr_slot[co, slot] = r[order].astype(np.float32)
    return idx_slot, dr_slot, chid_of_gc, blocks, NC, NSLOT


def _build_bass(blocks, NC, NSLOT):
    import concourse.bacc as bacc
    import concourse.tile as tile
    import concourse.mybir as mybir

    dt = mybir.dt
    NCP = -(-NC // 8) * 8
    nc = bacc.Bacc(
        "TRN2",
        target_bir_lowering=False,
        debug=False,
        num_devices=CORES,
        num_swdge_queues=NQ,
    )

    xs_in = nc.dram_tensor("xs", [NP, D], dt.float16, kind="ExternalInput")
    xself_in = nc.dram_tensor("xself", [SP, D], dt.float16, kind="ExternalInput")
    w1t_in = nc.dram_tensor("w1t", [D, D], dt.float16, kind="ExternalInput")
    w2t_in = nc.dram_tensor("w2t", [D, D], dt.float16, kind="ExternalInput")
    iota8_in = nc.dram_tensor("iota8", [128, 8 * 128], dt.float16, kind="ExternalInput")
    ident_in = nc.dram_tensor("ident", [128, 128], dt.float16, kind="ExternalInput")
    idx_in = nc.dram_tensor("idx", [128, NSLOT // 16], dt.int16, kind="ExternalInput")
    dr_in = nc.dram_tensor("dr", [128, NCP], dt.float16, kind="ExternalInput")
    dv1_in = nc.dram_tensor("dv1", [128, TPC], dt.float32, kind="ExternalInput")
    dv2_in = nc.dram_tensor("dv2", [128, TPC], dt.float32, kind="ExternalInput")
    c1d_in = nc.dram_tensor("c1d", [SP, D], dt.float16, kind="ExternalInput")
    c2_in = nc.dram_tensor("c2", [SP, D], dt.float32, kind="ExternalInput")
    out_ext = nc.dram_tensor("out", [SP, D], dt.float32, kind="ExternalOutput")

    GBASE = [i * GRPW for i in range(NGRP)]

    with tile.TileContext(nc) as tc:
        with (
            tc.tile_pool(name="const", bufs=1) as cpool,
            tc.tile_pool(name="mblk", bufs=2) as mpool,
            tc.tile_pool(name="selfp", bufs=2) as spool,
            tc.tile_pool(name="pbuf", bufs=6) as ppool,
            tc.tile_pool(name="drm", bufs=4) as dmpool,
            tc.tile_pool(name="gs", bufs=4) as gspool,
            tc.tile_pool(name="ytmp", bufs=4) as ytpool,
            tc.tile_pool(name="cblk", bufs=2) as clpool,
            tc.tile_pool(name="yblk", bufs=2) as ybpool,
            tc.tile_pool(name="psumG", bufs=4, space="PSUM") as pgpool,
            tc.tile_pool(name="psumH", bufs=4, space="PSUM") as phpool,
            tc.tile_pool(name="dram", bufs=1, space="DRAM") as dram_pool,
        ):
            iota8_t = cpool.tile([128, 8 * 128], dt.float16)
            nc.sync.dma_start(out=iota8_t[:], in_=iota8_in[:, :])
            ident_t = cpool.tile([128, 128], dt.float16)
            nc.sync.dma_start(out=ident_t[:], in_=ident_in[:, :])
            w1t_t = cpool.tile([D, D], dt.float16)
            nc.sync.dma_start(out=w1t_t[:], in_=w1t_in[:, :])
            w2t_t = cpool.tile([D, D], dt.float16)
            nc.sync.dma_start(out=w2t_t[:], in_=w2t_in[:, :])
            idx_t = cpool.tile([128, NSLOT // 16], dt.int16)
            nc.sync.dma_start(out=idx_t[:], in_=idx_in[:, :])
            dr_t = cpool.tile([128, NCP], dt.float16)
            nc.sync.dma_start(out=dr_t[:], in_=dr_in[:, :])
            dv1_t = cpool.tile([128, TPC], dt.float32)
            nc.sync.dma_start(out=dv1_t[:], in_=dv1_in[:, :])
            dv2_t = cpool.tile([128, TPC], dt.float32)
            nc.sync.dma_start(out=dv2_t[:], in_=dv2_in[:, :])

            y1_shard = dram_pool.tile([SP, D], dt.float16)
            y1_full = dram_pool.tile([NP, D], dt.float16)

            qctr = [0]

            def layer(src_dram, self_dram, c_dram, o_dram, wt_t, dv_t, last):
                ydt = dt.float32 if last else dt.float16
                cdt = dt.float32 if last else dt.float16
                pb_tiles = {}

                def get_p(chid):
                    bid = chid // 8
                    if bid not in pb_tiles:
                        # materialize the per-chunk dst-row broadcast on the
                        # (idle) Act engine so the Vector is_equal runs with
                        # unit-stride operands (2x DVE mode)
                        drm_t = dmpool.tile([128, 8, 128], dt.float16, tag="dm")
                        nc.scalar.copy(
                            out=drm_t[:],
                            in_=dr_t[:, bid * 8 : bid * 8 + 8, None].broadcast_to(
                                [128, 8, 128]
                            ),
                        )
                        pb_t = ppool.tile([128, 8, 128], dt.float16, tag="p")
                        nc.vector.tensor_tensor(
                            out=pb_t[:],
                            in0=iota8_t[:],
                            in1=drm_t[:],
                            op=mybir.AluOpType.is_equal,
                        )
                        pb_tiles[bid] = pb_t
                    return pb_tiles[bid][:, chid % 8, :]

                for bi, blk in enumerate(blocks):
                    TB = _tiles_in_block(bi)
                    base = bi * BLK * 128
                    rows = slice(base, base + TB * 128)
                    C = blk["C"]
                    m_t = mpool.tile([128, C, D], dt.float16, tag="m")
                    for gg, c0, c1, slot0, num in blk["gathers"]:
                        nc.gpsimd.dma_gather(
                            m_t[:, c0:c1, :],
                            src_dram[GBASE[gg] : GBASE[gg] + GRPW, :],
                            idx_t[:, slot0 // 16 : (slot0 + num) // 16],
                            num,
                            num,
                            D,
                            single_packet=False,
                            queue_num=qctr[0] % NQ,
                        )
                        qctr[0] += 1
                    self_t = spool.tile([128, TB * 128], dt.float16, tag="s")
                    nc.sync.dma_start(out=self_t[:], in_=self_dram[rows, :])
                    c_t = clpool.tile([128, TB * 128], cdt, tag=f"c{int(last)}")
                    nc.sync.dma_start(out=c_t[:], in_=c_dram[rows, :])
                    yb_t = ybpool.tile([128, TB * 128], ydt, tag=f"y{int(last)}")
                    for j, tt in enumerate(blk["tiles"]):
                        jcols = slice(j * 128, (j + 1) * 128)
                        chunks = blk["chunks"][tt]
                        psum_g = pgpool.tile([128, 128], dt.float32, space="PSUM")
                        nc.tensor.matmul(
                            psum_g[:],
                            lhsT=self_t[:, jcols],
                            rhs=ident_t[:],
                            start=True,
                            stop=(len(chunks) == 0),
                        )
                        for i, (col, chid) in enumerate(chunks):
                            nc.tensor.matmul(
                                psum_g[:],
                                lhsT=m_t[:, col, :],
                                rhs=get_p(chid),
                                start=False,
                                stop=(i == len(chunks) - 1),
                            )
                        gs_t = gspool.tile([128, 128], dt.float16, tag="gs")
                        nc.vector.tensor_copy(out=gs_t[:], in_=psum_g[:])
                        psum_h = phpool.tile([128, 128], dt.float32, space="PSUM")
                        nc.tensor.matmul(
                            psum_h[:], lhsT=gs_t[:], rhs=wt_t[:], start=True, stop=True
                        )
                        tmp_t = ytpool.tile([128, 128], ydt, tag=f"yt{int(last)}")
                        nc.vector.tensor_scalar(
                            out=tmp_t[:],
                            in0=psum_h[:],
                            scalar1=dv_t[:, tt : tt + 1],
                            scalar2=None,
                            op0=mybir.AluOpType.mult,
                        )
                        nc.vector.tensor_tensor(
                            out=yb_t[:, jcols],
                            in0=tmp_t[:],
                            in1=c_t[:, jcols],
                            op=mybir.AluOpType.add,
                        )
                    nc.sync.dma_start(out=o_dram[rows, :], in_=yb_t[:])

            layer(xs_in, xself_in, c1d_in, y1_shard, w1t_t, dv2_t, last=False)
            nc.gpsimd.collective_compute(
                "AllGather",
                mybir.AluOpType.bypass,
                replica_groups=[list(range(CORES))],
                ins=[y1_shard.opt()],
                outs=[y1_full.opt()],
            )
            layer(y1_full, y1_shard, c2_in, out_ext, w2t_t, dv1_t, last=True)

    nc.compile()
    return nc


def _prepare(x, edge_index, perturb_first, perturb_last, W1, b1, W2, b2):
    x = np.asarray(x, np.float32)
    edge_index = np.asarray(edge_index)
    src = edge_index[0].astype(np.int64)
    dst = edge_index[1].astype(np.int64)
    # degree includes self-loops (PyG adds one per node)
    deg = (np.bincount(dst, minlength=N) + 1).astype(np.float32)
    dinv = 1.0 / np.sqrt(deg)

    idx_slot, dr_slot, chid_of_gc, blocks, NC, NSLOT = _build_schedule(src, dst)
    NCP = -(-NC // 8) * 8
    perm = _bm_row(np.arange(S))  # local row l -> block-major row

    # gather source: xs = dinv * x, block-major shard layout
    xs = np.zeros((NP, D), np.float16)
    dinv_x = (dinv[:, None] * x).astype(np.float16)
    for c in range(CORES):
        xs[c * SP + perm] = dinv_x[c * S : (c + 1) * S]

    iota8 = np.broadcast_to(
        np.tile(np.arange(128, dtype=np.float16), 8), (128, 8 * 128)
    ).copy()
    ident = np.eye(128, dtype=np.float16)
    w1t = np.asarray(W1, np.float32).T.astype(np.float16).copy()
    w2t = np.asarray(W2, np.float32).T.astype(np.float16).copy()

    c1 = np.asarray(perturb_first, np.float32) + np.asarray(b1, np.float32)[None, :]
    c1d = dinv[:, None] * c1
    c2 = np.asarray(perturb_last, np.float32) + np.asarray(b2, np.float32)[None, :]

    in_maps = []
    for c in range(CORES):
        rows = slice(c * S, (c + 1) * S)
        c1d_p = np.zeros((SP, D), np.float16)
        c1d_p[perm] = c1d[rows].astype(np.float16)
        c2_p = np.zeros((SP, D), np.float32)
        c2_p[perm] = c2[rows]
        dv1 = np.zeros((TPC * 128,), np.float32)
        dv1[:S] = dinv[rows]
        idx_l = np.tile(idx_slot[c].reshape(-1, 16).T, (8, 1)).copy()
        dr_l = np.full((128, NCP), -1.0, np.float16)
        dr_l[:, chid_of_gc] = dr_slot[c].reshape(NC, 128).T
        in_maps.append(
            {
                "xs": xs,
                "xself": xs[c * SP : (c + 1) * SP],
                "w1t": w1t,
                "w2t": w2t,
                "iota8": iota8,
                "ident": ident,
                "idx": idx_l,
                "dr": dr_l,
                "dv1": np.ascontiguousarray(dv1.reshape(TPC, 128).T),
                "dv2": np.ascontiguousarray((dv1 * dv1).reshape(TPC, 128).T),
                "c1d": c1d_p,
                "c2": c2_p,
            }
        )
    return in_maps, blocks, NC, NSLOT


def kernel(x, edge_index, perturb_first, perturb_last, W1, b1, W2, b2, _results=[]):
    from concourse.bass_utils import run_bass_kernel_spmd

    in_maps, blocks, NC, NSLOT = _prepare(
        x, edge_index, perturb_first, perturb_last, W1, b1, W2, b2
    )
    nc = _build_bass(blocks, NC, NSLOT)
    res = run_bass_kernel_spmd(nc, in_maps, core_ids=list(range(CORES)))
    _results.append(res)
    perm = _bm_row(np.arange(S))
    out = np.concatenate(
        [res.results[c]["out"][perm] for c in range(CORES)], axis=0
    )
    return out.astype(np.float32)


# revision 16
# speedup vs baseline: 1.0572x; 1.0572x over previous
"""Trainium2 Bass kernel for a 2-layer GCN (PyG GCNConv semantics) on 8 NeuronCores.

Strategy (dst-sharding):
  - nodes sharded 12500/core (padded to 12544 = 98*128 tiles of 128 rows)
  - all node-row tensors live in DRAM in a block-major layout (row
    bm = block_base + r*TB + j for tile j of the block): per-BLOCK 256KB
    contiguous dma_starts replace per-tile 32KB ones, and gather indices
    are emitted against the same layout
  - edges (self-loops excluded) partitioned by destination core; within a
    core grouped by (dst-tile, src-group) where a src group is 2 source
    cores (25088 rows < int16 gather-index limit), padded to 128-edge chunks
  - per chunk: dma_gather of 128 source rows (fp16, round-robin over 4
    SWDGE queues) + pure one-hot P[t,r]=(iota==dr) built 8 chunks per
    tensor_tensor (broadcast-AP read of per-chunk dst-row columns); a PE
    matmul accumulates agg^T into PSUM
  - self-loop term: per-block contiguous load + per-tile matmul against a
    constant identity (no gather rows spent on it)
  - per dst-tile: agg^T @ W^T flips orientation back to [row, feat];
    dinv[dst] (squared for layer 1, which also pre-scales the next layer's
    gather source) is applied in the epilogue as a per-partition scalar
  - layer 1 -> AllGather of y1 shards (block-major) -> layer 2
"""
import sys

sys.path.insert(0, "/opt/trn_rl_repo")

import numpy as np

N = 100000
E = 1600000
D = 128
CORES = 8
S = 12500          # real nodes per core
TPC = 98           # dst tiles per core
SP = TPC * 128     # padded nodes per core (12544)
NP = CORES * SP    # padded global rows (100352)
NGRP = 4           # src groups = pairs of cores
GRPW = 2 * SP      # rows per src group (25088 < 32768 int16 limit)
BLK = 8            # dst tiles per block
NB = (TPC + BLK - 1) // BLK  # 13 blocks (12 full + 1 of 2 tiles)
GSPLIT = 24        # max columns (128-idx chunks) per gather instruction
NQ = 4             # SWDGE queues, gathers round-robin across them


def _tiles_in_block(b):
    return BLK if b < TPC // BLK else TPC - (TPC // BLK) * BLK


def _bm_row(l):
    """Local row index -> block-major row index (vectorized)."""
    l = np.asarray(l)
    t = l >> 7
    r = l & 127
    b = t // BLK
    j = t - b * BLK
    tb = np.where(b < TPC // BLK, BLK, TPC - (TPC // BLK) * BLK)
    return b * BLK * 128 + r * tb + j


def _build_schedule(src, dst):
    """Static chunk schedule shared by all cores (SPMD: one instruction
    stream). Returns per-core slot arrays + the chunk/block layout."""
    core = dst // S
    dl = dst % S
    t = dl >> 7
    r = dl & 127
    score = src // S
    g = score // 2
    srel = (score % 2) * SP + _bm_row(src % S)

    key = (core * TPC + t) * NGRP + g
    order = np.argsort(key, kind="stable")
    cnt = np.bincount(key, minlength=CORES * TPC * NGRP).reshape(CORES, TPC, NGRP)
    K = -(-cnt.max(0) // 128)  # [TPC, NGRP] chunks per (tile, group)

    # gather/slot layout order: for b in blocks: for g: for t in b: for k in
    # K[t,g].  Chunk IDs (dr-table columns, P-build batches) are assigned
    # separately in CONSUMPTION order (for b: for t: for g: for k) so the
    # lazily-built 8-chunk P batches are created and retired in the same
    # order PE consumes them — a first-use ordering would invert against
    # the in-order Vector queue and deadlock the tile-pool rings.
    chunk_start = np.zeros((TPC, NGRP), np.int64)
    blocks = []
    nchunks = 0
    raw_chunks = {}  # (t, g, k) -> gather-order chunk index (slot base / m col)
    for b in range(NB):
        tiles = list(range(b * BLK, min((b + 1) * BLK, TPC)))
        col = 0
        gathers = []
        for gg in range(NGRP):
            c0 = col
            slot0 = nchunks * 128
            for tt in tiles:
                chunk_start[tt, gg] = nchunks
                for k in range(int(K[tt, gg])):
                    raw_chunks[(tt, gg, k)] = (col, nchunks)
                    col += 1
                    nchunks += 1
            # split into <=GSPLIT-column pieces: keeps descriptor counts per
            # instruction low (ring holds 1024/direction) and lets pieces
            # round-robin across the SWDGE queues for DMA parallelism
            c_at = c0
            s_at = slot0
            while c_at < col:
                ncols = min(col - c_at, GSPLIT)
                gathers.append((gg, c_at, c_at + ncols, s_at, ncols * 128))
                c_at += ncols
                s_at += ncols * 128
        blocks.append(dict(tiles=tiles, C=col, gathers=gathers))
    NC = nchunks
    NSLOT = NC * 128

    # consumption-ordered chunk ids + gather-chunk -> chid permutation
    chid_of_gc = np.zeros(NC, np.int64)
    cid = 0
    for b, blk in enumerate(blocks):
        tile_chunks = {}
        for tt in blk["tiles"]:
            lst = []
            for gg in range(NGRP):
                for k in range(int(K[tt, gg])):
                    col, gc = raw_chunks[(tt, gg, k)]
                    chid_of_gc[gc] = cid
                    lst.append((col, cid))
                    cid += 1
            tile_chunks[tt] = lst
        blk["chunks"] = tile_chunks

    # per-core slot arrays
    skey = key[order]
    runs = np.flatnonzero(np.diff(skey)) + 1
    starts = np.r_[0, runs]
    lens = np.diff(np.r_[starts, len(skey)])
    pos = np.arange(len(skey)) - np.repeat(starts, lens)
    slot = chunk_start[t[order], g[order]] * 128 + pos

    idx_slot = np.zeros((CORES, NSLOT), np.int16)
    dr_slot = np.full((CORES, NSLOT), -1.0, np.float32)
    co = core[order]
    idx_slot[co, slot] = srel[order].astype(np.int16)
    dr_slot[co, slot] = r[order].astype(np.float32)
    return idx_slot, dr_slot, chid_of_gc, blocks, NC, NSLOT


def _build_bass(blocks, NC, NSLOT):
    import concourse.bacc as bacc
    import concourse.tile as tile
    import concourse.mybir as mybir

    dt = mybir.dt
    NCP = -(-NC // 8) * 8
    nc = bacc.Bacc(
        "TRN2",
        target_bir_lowering=False,
        debug=False,
        num_devices=CORES,
        num_swdge_queues=NQ,
    )

    xs_in = nc.dram_tensor("xs", [NP, D], dt.float16, kind="ExternalInput")
    xself_in = nc.dram_tensor("xself", [SP, D], dt.float16, kind="ExternalInput")
    w1t_in = nc.dram_tensor("w1t", [D, D], dt.float16, kind="ExternalInput")
    w2t_in = nc.dram_tensor("w2t", [D, D], dt.float16, kind="ExternalInput")
    iota8_in = nc.dram_tensor("iota8", [128, 8 * 128], dt.float16, kind="ExternalInput")
    ident_in = nc.dram_tensor("ident", [128, 128], dt.float16, kind="ExternalInput")
    idx_in = nc.dram_tensor("idx", [128, NSLOT // 16], dt.int16, kind="ExternalInput")
    dr_in = nc.dram_tensor("dr", [128, NCP], dt.float16, kind="ExternalInput")
    dv1_in = nc.dram_tensor("dv1", [128, TPC], dt.float32, kind="ExternalInput")
    dv2_in = nc.dram_tensor("dv2", [128, TPC], dt.float32, kind="ExternalInput")
    c1d_in = nc.dram_tensor("c1d", [SP, D], dt.float16, kind="ExternalInput")
    c2_in = nc.dram_tensor("c2", [SP, D], dt.float32, kind="ExternalInput")
    out_ext = nc.dram_tensor("out", [SP, D], dt.float32, kind="ExternalOutput")

    GBASE = [i * GRPW for i in range(NGRP)]

    with tile.TileContext(nc) as tc:
        with (
            tc.tile_pool(name="const", bufs=1) as cpool,
            tc.tile_pool(name="mblk", bufs=2) as mpool,
            tc.tile_pool(name="selfp", bufs=2) as spool,
            tc.tile_pool(name="pbuf", bufs=6) as ppool,
            tc.tile_pool(name="drm", bufs=4) as dmpool,
            tc.tile_pool(name="gs", bufs=4) as gspool,
            tc.tile_pool(name="ytmp", bufs=4) as ytpool,
            tc.tile_pool(name="cblk", bufs=2) as clpool,
            tc.tile_pool(name="yblk", bufs=2) as ybpool,
            tc.tile_pool(name="psumG", bufs=4, space="PSUM") as pgpool,
            tc.tile_pool(name="psumH", bufs=4, space="PSUM") as phpool,
            tc.tile_pool(name="dram", bufs=1, space="DRAM") as dram_pool,
        ):
            iota8_t = cpool.tile([128, 8 * 128], dt.float16)
            nc.sync.dma_start(out=iota8_t[:], in_=iota8_in[:, :])
            ident_t = cpool.tile([128, 128], dt.float16)
            nc.sync.dma_start(out=ident_t[:], in_=ident_in[:, :])
            w1t_t = cpool.tile([D, D], dt.float16)
            nc.sync.dma_start(out=w1t_t[:], in_=w1t_in[:, :])
            w2t_t = cpool.tile([D, D], dt.float16)
            nc.sync.dma_start(out=w2t_t[:], in_=w2t_in[:, :])
            idx_t = cpool.tile([128, NSLOT // 16], dt.int16)
            nc.sync.dma_start(out=idx_t[:], in_=idx_in[:, :])
            dr_t = cpool.tile([128, NCP], dt.float16)
            nc.sync.dma_start(out=dr_t[:], in_=dr_in[:, :])
            dv1_t = cpool.tile([128, TPC], dt.float32)
            nc.sync.dma_start(out=dv1_t[:], in_=dv1_in[:, :])
            dv2_t = cpool.tile([128, TPC], dt.float32)
            nc.sync.dma_start(out=dv2_t[:], in_=dv2_in[:, :])

            y1_shard = dram_pool.tile([SP, D], dt.float16)
            y1_full = dram_pool.tile([NP, D], dt.float16)

            qctr = [0]

            def layer(src_dram, self_dram, c_dram, o_dram, wt_t, dv_t, last):
                ydt = dt.float32 if last else dt.float16
                cdt = dt.float32 if last else dt.float16
                pb_tiles = {}

                def get_p(chid):
                    bid = chid // 8
                    if bid not in pb_tiles:
                        # materialize the per-chunk dst-row broadcast on the
                        # (idle) Act engine so the Vector is_equal runs with
                        # unit-stride operands (2x DVE mode)
                        drm_t = dmpool.tile([128, 8, 128], dt.float16, tag="dm")
                        nc.scalar.copy(
                            out=drm_t[:],
                            in_=dr_t[:, bid * 8 : bid * 8 + 8, None].broadcast_to(
                                [128, 8, 128]
                            ),
                        )
                        pb_t = ppool.tile([128, 8, 128], dt.float16, tag="p")
                        nc.vector.tensor_tensor(
                            out=pb_t[:],
                            in0=iota8_t[:],
                            in1=drm_t[:],
                            op=mybir.AluOpType.is_equal,
                        )
                        pb_tiles[bid] = pb_t
                    return pb_tiles[bid][:, chid % 8, :]

                for bi, blk in enumerate(blocks):
                    TB = _tiles_in_block(bi)
                    base = bi * BLK * 128
                    rows = slice(base, base + TB * 128)
                    C = blk["C"]
                    m_t = mpool.tile([128, C, D], dt.float16, tag="m")
                    for gg, c0, c1, slot0, num in blk["gathers"]:
                        nc.gpsimd.dma_gather(
                            m_t[:, c0:c1, :],
                            src_dram[GBASE[gg] : GBASE[gg] + GRPW, :],
                            idx_t[:, slot0 // 16 : (slot0 + num) // 16],
                            num,
                            num,
                            D,
                            single_packet=False,
                            queue_num=qctr[0] % NQ,
                        )
                        qctr[0] += 1
                    self_t = spool.tile([128, TB * 128], dt.float16, tag="s")
                    nc.sync.dma_start(out=self_t[:], in_=self_dram[rows, :])
                    c_t = clpool.tile([128, TB * 128], cdt, tag=f"c{int(last)}")
                    nc.sync.dma_start(out=c_t[:], in_=c_dram[rows, :])
                    yb_t = ybpool.tile([128, TB * 128], ydt, tag=f"y{int(last)}")
                    for j, tt in enumerate(blk["tiles"]):
                        jcols = slice(j * 128, (j + 1) * 128)
                        chunks = blk["chunks"][tt]
                        psum_g = pgpool.tile([128, 128], dt.float32, space="PSUM")
                        nc.tensor.matmul(
                            psum_g[:],
                            lhsT=self_t[:, jcols],
                            rhs=ident_t[:],
                            start=True,
                            stop=(len(chunks) == 0),
                        )
                        for i, (col, chid) in enumerate(chunks):
                            nc.tensor.matmul(
                                psum_g[:],
                                lhsT=m_t[:, col, :],
                                rhs=get_p(chid),
                                start=False,
                                stop=(i == len(chunks) - 1),
                            )
                        gs_t = gspool.tile([128, 128], dt.float16, tag="gs")
                        nc.vector.tensor_copy(out=gs_t[:], in_=psum_g[:])
                        psum_h = phpool.tile([128, 128], dt.float32, space="PSUM")
                        nc.tensor.matmul(
                            psum_h[:], lhsT=gs_t[:], rhs=wt_t[:], start=True, stop=True
                        )
                        tmp_t = ytpool.tile([128, 128], ydt, tag=f"yt{int(last)}")
                        nc.vector.tensor_scalar(
                            out=tmp_t[:],
                            in0=psum_h[:],
                            scalar1=dv_t[:, tt : tt + 1],
                            scalar2=None,
                            op0=mybir.AluOpType.mult,
                        )
                        nc.vector.tensor_tensor(
                            out=yb_t[:, jcols],
                            in0=tmp_t[:],
                            in1=c_t[:, jcols],
                            op=mybir.AluOpType.add,
                        )
                    nc.sync.dma_start(out=o_dram[rows, :], in_=yb_t[:])

            layer(xs_in, xself_in, c1d_in, y1_shard, w1t_t, dv2_t, last=False)
            nc.gpsimd.collective_compute(
                "AllGather",
                mybir.AluOpType.bypass,
                replica_groups=[list(range(CORES))],
                ins=[y1_shard.opt()],
                outs=[y1_full.opt()],
            )
            layer(y1_full, y1_shard, c2_in, out_ext, w2t_t, dv1_t, last=True)

    nc.compile()
    return nc


def _prepare(x, edge_index, perturb_first, perturb_last, W1, b1, W2, b2):
    x = np.asarray(x, np.float32)
    edge_index = np.asarray(edge_index)
    src = edge_index[0].astype(np.int64)
    dst = edge_index[1].astype(np.int64)
    # degree includes self-loops (PyG adds one per node)
    deg = (np.bincount(dst, minlength=N) + 1).astype(np.float32)
    dinv = 1.0 / np.sqrt(deg)

    idx_slot, dr_slot, chid_of_gc, blocks, NC, NSLOT = _build_schedule(src, dst)
    NCP = -(-NC // 8) * 8
    perm = _bm_row(np.arange(S))  # local row l -> block-major row

    # gather source: xs = dinv * x, block-major shard layout
    xs = np.zeros((NP, D), np.float16)
    dinv_x = (dinv[:, None] * x).astype(np.float16)
    for c in range(CORES):
        xs[c * SP + perm] = dinv_x[c * S : (c + 1) * S]

    iota8 = np.broadcast_to(
        np.tile(np.arange(128, dtype=np.float16), 8), (128, 8 * 128)
    ).copy()
    ident = np.eye(128, dtype=np.float16)
    w1t = np.asarray(W1, np.float32).T.astype(np.float16).copy()
    w2t = np.asarray(W2, np.float32).T.astype(np.float16).copy()

    c1 = np.asarray(perturb_first, np.float32) + np.asarray(b1, np.float32)[None, :]
    c1d = dinv[:, None] * c1
    c2 = np.asarray(perturb_last, np.float32) + np.asarray(b2, np.float32)[None, :]

    in_maps = []
    for c in range(CORES):
        rows = slice(c * S, (c + 1) * S)
        c1d_p = np.zeros((SP, D), np.float16)
        c1d_p[perm] = c1d[rows].astype(np.float16)
        c2_p = np.zeros((SP, D), np.float32)
        c2_p[perm] = c2[rows]
        dv1 = np.zeros((TPC * 128,), np.float32)
        dv1[:S] = dinv[rows]
        idx_l = np.tile(idx_slot[c].reshape(-1, 16).T, (8, 1)).copy()
        dr_l = np.full((128, NCP), -1.0, np.float16)
        dr_l[:, chid_of_gc] = dr_slot[c].reshape(NC, 128).T
        in_maps.append(
            {
                "xs": xs,
                "xself": xs[c * SP : (c + 1) * SP],
                "w1t": w1t,
                "w2t": w2t,
                "iota8": iota8,
                "ident": ident,
                "idx": idx_l,
                "dr": dr_l,
                "dv1": np.ascontiguousarray(dv1.reshape(TPC, 128).T),
                "dv2": np.ascontiguousarray((dv1 * dv1).reshape(TPC, 128).T),
                "c1d": c1d_p,
                "c2": c2_p,
            }
        )
    return in_maps, blocks, NC, NSLOT


def kernel(x, edge_index, perturb_first, perturb_last, W1, b1, W2, b2, _results=[]):
    from concourse.bass_utils import run_bass_kernel_spmd

    in_maps, blocks, NC, NSLOT = _prepare(
        x, edge_index, perturb_first, perturb_last, W1, b1, W2, b2
    )
    nc = _build_bass(blocks, NC, NSLOT)
    res = run_bass_kernel_spmd(nc, in_maps, core_ids=list(range(CORES)))
    _results.append(res)
    perm = _bm_row(np.arange(S))
    out = np.concatenate(
        [res.results[c]["out"][perm] for c in range(CORES)], axis=0
    )
    return out.astype(np.float32)
